# revision 28
# baseline (speedup 1.0000x reference)
"""FCOS multi-stride filter kernel for Trainium2 (8 NeuronCores, batch-parallel).

Reference computation (per level l with stride s, per batch b):
    scores  = cls[b].reshape(C, H*W)          # natural NCHW layout
    mask    = (max_c scores > 0.5)            # per location
    out[loc] = [x*mask, y*mask, scores*mask, bbox*mask, ctr*mask]  # [HW, 87]

Strategy:
  - Host packs, per core (= batch sample), one [87, 34100] f32 tensor:
    rows 0:2 = precomputed pixel coords, 2:82 = cls, 82:86 = bbox, 86 = ctr;
    columns = concatenated locations of all 5 pyramid levels.
  - Device (per core): for each block of up to 512 locations, 4 PE transposes
    (stride-4 interleave so each output partition holds 4 consecutive
    locations = 1392B contiguous DRAM runs), DVE reduce_max + threshold
    compare for the mask, ScalarE/VectorE masked copies PSUM->SBUF with
    per-partition scale, then one DMA of [P, 348] to the output.
  - Output per core is [34100, 87]; host splits rows per level and stacks
    over the 8 cores.
"""

import numpy as np

import concourse.bass as bass
import concourse.tile as tile
from concourse import mybir

# Problem constants (hardcoded per harness contract)
B = 8
C = 80
HS = (160, 80, 40, 20, 10)
STRIDES = (8, 16, 32, 64, 128)
THRESHOLD = 0.5

HWS = [h * h for h in HS]
TOTAL = sum(HWS)  # 34100
LEVEL_OFF = [0]
for _hw in HWS:
    LEVEL_OFF.append(LEVEL_OFF[-1] + _hw)

NCH = 87  # 2 coord + 80 cls + 4 bbox + 1 ctr
NR = 88  # packed input rows (87 channels + 1 zero row so K=88 transposes are
         # true permutations for CoreSim and the psum pad column gets 0.0)
GP = 88  # per-group column pitch in psum/out tiles (pad to even for DVE 2x)
GW = 4 * GP  # 352 floats per psum/out tile row (4 locations x 88)
CHUNK = 4096  # input DMA chunk (columns)
F32 = mybir.dt.float32


def _blocks():
    """Yield (chunk_start, chunk_len, [group, ...]); each group is a list of
    consecutive (block_start, block_len) blocks sharing one output staging
    tile and one batched output DMA (runs of full 512-blocks, or one tail).

    Blocks are <=512 locations, never cross a chunk or level boundary, and
    are always divisible by 4.
    """
    chunk_bounds = list(range(0, TOTAL, CHUNK)) + [TOTAL]
    out = []
    for ci in range(len(chunk_bounds) - 1):
        c0, c1 = chunk_bounds[ci], chunk_bounds[ci + 1]
        groups = []
        for li in range(5):
            l0, l1 = LEVEL_OFF[li], LEVEL_OFF[li + 1]
            s = max(c0, l0)
            e = min(c1, l1)
            b0 = s
            run = []
            while b0 < e:
                bn = min(512, e - b0)
                assert bn % 4 == 0
                if bn == 512:
                    run.append((b0, bn))
                else:
                    if run:
                        groups.append(run)
                        run = []
                    groups.append([(b0, bn)])
                b0 += bn
            if run:
                groups.append(run)
        if groups:
            out.append((c0, c1 - c0, groups))
    return out


def _emit(tc, outs, ins):
    """Emit the per-core Bass program under a TileContext."""
    nc = tc.nc
    pk = ins["pk"]        # [87, TOTAL] dram
    ident = ins["ident"]  # [87, 87] dram
    out_d = outs["out"]   # [TOTAL, 87] dram

    NPS = 4  # psum ping-pong depth (deterministic tenancy: block i evicts i-4)

    with (
        tc.tile_pool(name="const", bufs=1) as cpool,
        tc.tile_pool(name="inp", bufs=3) as ipool,
        tc.tile_pool(name="stg", bufs=3) as stpool,
        tc.tile_pool(name="small", bufs=8) as spool,
    ):
        pspools = [
            tc.alloc_tile_pool(name=f"ps{i}", bufs=1, space="PSUM")
            for i in range(NPS)
        ]
        id_t = cpool.tile([NR, GP], F32)
        nc.sync.dma_start(out=id_t[:, :], in_=ident[:, :])

        bi = 0
        ci = 0
        gi = 0
        for c0, cn, groups in _blocks():
            in_t = ipool.tile([NR, cn], F32, tag="in")
            # Spread input chunks over the SP HWDGE ring and the SWDGE
            # (gpsimd) ring so no single descriptor ring serializes all
            # transfers; ACT's ring is kept free for compute.
            in_eng = nc.gpsimd if ci in (0, 4, 8) else nc.sync
            in_eng.dma_start(out=in_t[:, :], in_=pk[:, c0 : c0 + cn])
            ci += 1

            for group in groups:
                L = len(group)
                P = group[0][1] // 4
                g0 = group[0][0]
                # Dense staging: location (g0 + 4P*j + 4p + q), channel c at
                # stage[p, 348*j + 87*q + c] -- contiguous 1392B runs that
                # batch L blocks into ONE output DMA (3-dim balanced APs).
                stage = stpool.tile([P, 348 * L], F32, tag="st")
                for j, (b0, bn) in enumerate(group):
                    ps_t = pspools[bi % NPS].tile(
                        [P, GW], F32, tag="ps", name=f"ps_t{bi % NPS}"
                    )
                    # [NR, bn] -> [NR, 4, P] view: location = b0 + 4*p + q
                    # The identity is the [88, 88] permutation; input row 87
                    # is zero, so each transpose writes the full 88-wide
                    # group (pad col = 0.0): no uninitialized PSUM reads.
                    src = in_t[:, b0 - c0 : b0 - c0 + bn].rearrange(
                        "c (p k) -> c k p", k=4
                    )
                    for q in range(4):
                        nc.tensor.transpose(
                            ps_t[:, GP * q : GP * (q + 1)],
                            src[:, q, :],
                            id_t[:, :],
                        )
                    # mask = (max over class channels) > THRESHOLD
                    maxv = spool.tile([P, 4], F32, tag="mx")
                    red_in = ps_t[:, :].rearrange("p (g c) -> p g c", c=GP)[
                        :, :, 2:82
                    ]
                    nc.vector.reduce_max(
                        maxv[:, :], red_in, axis=mybir.AxisListType.X
                    )
                    mask = spool.tile([P, 4], F32, tag="mk")
                    nc.vector.tensor_scalar(
                        mask[:, :], maxv[:, :], THRESHOLD, None,
                        mybir.AluOpType.is_gt,
                    )
                    # Masked copies PSUM->dense staging (mask fused as the
                    # per-partition activation scale); 3 on ACT, 1 on DVE.
                    for q in range(4):
                        dst = stage[:, 348 * j + 87 * q : 348 * j + 87 * (q + 1)]
                        sq = ps_t[:, GP * q : GP * q + NCH]
                        mq = mask[:, q : q + 1]
                        if q == 3:
                            nc.vector.tensor_scalar(
                                dst, sq, mq, None, mybir.AluOpType.mult
                            )
                        else:
                            nc.scalar.mul(dst, sq, mq)
                    bi += 1
                # One batched output DMA per group, alternating rings.
                dram_view = out_d[g0 : g0 + 4 * P * L, :].rearrange(
                    "(j p k) c -> p j (k c)", j=L, k=4
                )
                sb_view = stage[:, :].rearrange("p (j x) -> p j x", x=348)
                out_eng = nc.gpsimd if (gi % 2) else nc.sync
                out_eng.dma_start(out=dram_view, in_=sb_view)
                gi += 1
        for p in reversed(pspools):
            p.release()


_NC_CACHE = None


def _split_multi_waits(nc):
    """Hoist all-but-one semaphore wait of every instruction into preceding
    single-wait NoOps on the same engine.

    The TPB instruction encoding has ONE sync-wait slot per instruction and
    this walrus build refuses to split multi-wait instructions itself
    ("Too many sync wait commands"). Engines process their instruction
    stream in order, so waiting via a chain of NoOps is equivalent.
    """
    f = nc.m.functions[0]
    for blk in f.blocks:
        targets = []
        for idx, inst in enumerate(blk.instructions):
            si = inst.sync_info
            if si is not None and len(si.on_wait) >= 2:
                targets.append((idx, inst))
        for idx, inst in reversed(targets):
            si = inst.sync_info
            waits = list(si.on_wait)
            inst.sync_info = mybir.SyncInfo(
                on_wait=[waits[-1]], on_update=list(si.on_update)
            )
            for w in reversed(waits[:-1]):
                nop = mybir.InstNoOp(
                    name=nc.get_next_instruction_name(),
                    bass_nofuse=True,
                    sync_info=mybir.SyncInfo(on_wait=[w], on_update=[]),
                )
                nop.engine = inst.engine
                nc.register_instruction(nop)
                blk.instructions.insert(idx, nop)


def _build_nc(reps=1):
    """Build the Bass program; reps>1 repeats the whole (idempotent) body —
    used only for marginal-time measurement in the test harness."""
    nc = bass.Bass("TRN2", target_bir_lowering=False, debug=False)
    pk = nc.dram_tensor("pk", [NR, TOTAL], F32, kind="ExternalInput")
    ident = nc.dram_tensor("ident", [NR, GP], F32, kind="ExternalInput")
    out = nc.dram_tensor("out", [TOTAL, 87], F32, kind="ExternalOutput")
    with tile.TileContext(nc) as tc:
        for _ in range(reps):
            _emit(tc, {"out": out[:, :]}, {"pk": pk[:, :], "ident": ident[:, :]})
    _split_multi_waits(nc)
    return nc


def _get_nc():
    global _NC_CACHE
    if _NC_CACHE is None:
        _NC_CACHE = _build_nc()
    return _NC_CACHE


def _coord_rows(h, stride):
    half = stride // 2
    xs = np.tile(np.arange(h, dtype=np.float32) * stride + half, h)
    ys = np.repeat(np.arange(h, dtype=np.float32) * stride + half, h)
    return xs, ys


def pack_inputs(inputs):
    """Build per-core input maps from the full (unsharded) input dict."""
    # [88, 88] permutation: transposes write the full 88-wide psum group;
    # input row 87 is zero, so the psum pad column lands as 0.0.
    ident = np.eye(GP, dtype=np.float32)
    # Per-level coordinate rows are identical across cores; compute once.
    coords = [_coord_rows(h, s) for h, s in zip(HS, STRIDES)]
    in_maps = []
    for b in range(B):
        pk = np.zeros((NR, TOTAL), np.float32)
        for li, h in enumerate(HS):
            o, hw = LEVEL_OFF[li], HWS[li]
            sl = slice(o, o + hw)
            pk[0, sl], pk[1, sl] = coords[li]
            pk[2:82, sl] = np.asarray(inputs[f"cls_{li}"][b]).reshape(C, hw)
            pk[82:86, sl] = np.asarray(inputs[f"bbox_{li}"][b]).reshape(4, hw)
            pk[86, sl] = np.asarray(inputs[f"ctr_{li}"][b]).reshape(hw)
        in_maps.append({"pk": pk, "ident": ident})
    return in_maps


def unpack_outputs(results):
    """results: list (per core) of {name: np.ndarray}. Returns tuple of 5."""
    outs = []
    for li, hw in enumerate(HWS):
        o = LEVEL_OFF[li]
        outs.append(
            np.stack([results[b]["out"][o : o + hw, :] for b in range(B)], axis=0)
        )
    return tuple(outs)


def kernel(**inputs):
    from concourse.bass_utils import run_bass_kernel_spmd

    in_maps = pack_inputs(inputs)
    nc = _get_nc()
    res = run_bass_kernel_spmd(nc, in_maps, core_ids=list(range(B)))
    return unpack_outputs(res.results)


# revision 30
# speedup vs baseline: 16.3816x; 16.3816x over previous
"""FCOS multi-stride filter kernel for Trainium2 (8 NeuronCores, batch-parallel).

Reference computation (per level l with stride s, per batch b):
    scores  = cls[b].reshape(C, H*W)          # natural NCHW layout
    mask    = (max_c scores > 0.5)            # per location
    out[loc] = [x*mask, y*mask, scores*mask, bbox*mask, ctr*mask]  # [HW, 87]

Strategy:
  - Host packs, per core (= batch sample), one [88, 34100] f32 tensor:
    rows 0:2 = precomputed pixel coords, 2:82 = cls, 82:86 = bbox, 86 = ctr,
    87 = zeros; columns = concatenated locations of all 5 pyramid levels.
  - Device (per core): for each block of 512 locations, 4 PE transposes
    (stride-4 interleave so each output partition holds 4 consecutive
    locations), DVE reduce_max + threshold compare for the mask, then 4
    masked copies (mask fused as the per-partition activation/tensor-scalar
    scale) from PSUM into a DENSE staging tile whose rows are 348-float
    contiguous runs. Up to 8 blocks share one staging tile and ONE ~1.4MB
    output DMA (26 DMAs total per core instead of 79 - per-DMA fixed cost
    dominated the measured runtime).
  - Output per core is [34100, 87]; host splits rows per level and stacks
    over the 8 cores.
"""

import numpy as np

import concourse.bass as bass
import concourse.tile as tile
from concourse import mybir

# Problem constants (hardcoded per harness contract)
B = 8
C = 80
HS = (160, 80, 40, 20, 10)
STRIDES = (8, 16, 32, 64, 128)
THRESHOLD = 0.5

HWS = [h * h for h in HS]
TOTAL = sum(HWS)  # 34100
LEVEL_OFF = [0]
for _hw in HWS:
    LEVEL_OFF.append(LEVEL_OFF[-1] + _hw)

NCH = 87  # 2 coord + 80 cls + 4 bbox + 1 ctr
NR = 88  # packed input rows (87 channels + 1 zero row so K=88 transposes are
         # true permutations for CoreSim and the psum pad column gets 0.0)
GP = 88  # per-group column pitch in psum/out tiles (pad to even for DVE 2x)
GW = 4 * GP  # 352 floats per psum/out tile row (4 locations x 88)
CHUNK = 8192  # input DMA chunk (columns)
F32 = mybir.dt.float32


def _blocks():
    """Yield (chunk_start, chunk_len, [group, ...]); each group is a list of
    consecutive (block_start, block_len) blocks sharing one output staging
    tile and one batched output DMA (runs of full 512-blocks, or one tail).

    Blocks are <=512 locations, never cross a chunk or level boundary, and
    are always divisible by 4.
    """
    chunk_bounds = list(range(0, TOTAL, CHUNK)) + [TOTAL]
    out = []
    for ci in range(len(chunk_bounds) - 1):
        c0, c1 = chunk_bounds[ci], chunk_bounds[ci + 1]
        groups = []
        for li in range(5):
            l0, l1 = LEVEL_OFF[li], LEVEL_OFF[li + 1]
            s = max(c0, l0)
            e = min(c1, l1)
            b0 = s
            run = []
            while b0 < e:
                bn = min(512, e - b0)
                assert bn % 4 == 0
                if bn == 512:
                    run.append((b0, bn))
                else:
                    if run:
                        groups.append(run)
                        run = []
                    groups.append([(b0, bn)])
                b0 += bn
            if run:
                groups.append(run)
        if groups:
            out.append((c0, c1 - c0, groups))
    return out


def _emit(tc, outs, ins):
    """Emit the per-core Bass program under a TileContext."""
    nc = tc.nc
    pk = ins["pk"]        # [87, TOTAL] dram
    ident = ins["ident"]  # [87, 87] dram
    out_d = outs["out"]   # [TOTAL, 87] dram

    NPS = 4  # psum ping-pong depth (deterministic tenancy: block i evicts i-4)

    with (
        tc.tile_pool(name="const", bufs=1) as cpool,
        tc.tile_pool(name="inp", bufs=2) as ipool,
        tc.tile_pool(name="stg", bufs=3) as stpool,
        tc.tile_pool(name="small", bufs=8) as spool,
    ):
        pspools = [
            tc.alloc_tile_pool(name=f"ps{i}", bufs=1, space="PSUM")
            for i in range(NPS)
        ]
        id_t = cpool.tile([NR, GP], F32)
        nc.sync.dma_start(out=id_t[:, :], in_=ident[:, :])

        bi = 0
        ci = 0
        gi = 0
        for c0, cn, groups in _blocks():
            in_t = ipool.tile([NR, cn], F32, tag="in")
            # Spread input chunks over the SP HWDGE ring and the SWDGE
            # (gpsimd) ring so no single descriptor ring serializes all
            # transfers; ACT's ring is kept free for compute.
            in_eng = nc.gpsimd if ci in (0, 3) else nc.sync
            in_eng.dma_start(out=in_t[:, :], in_=pk[:, c0 : c0 + cn])
            ci += 1

            for group in groups:
                L = len(group)
                P = group[0][1] // 4
                g0 = group[0][0]
                # Dense staging: location (g0 + 4P*j + 4p + q), channel c at
                # stage[p, 348*j + 87*q + c] -- contiguous 1392B runs that
                # batch L blocks into ONE output DMA (3-dim balanced APs).
                stage = stpool.tile([P, 348 * L], F32, tag="st")
                for j, (b0, bn) in enumerate(group):
                    ps_t = pspools[bi % NPS].tile(
                        [P, GW], F32, tag="ps", name=f"ps_t{bi % NPS}"
                    )
                    # [NR, bn] -> [NR, 4, P] view: location = b0 + 4*p + q
                    # The identity is the [88, 88] permutation; input row 87
                    # is zero, so each transpose writes the full 88-wide
                    # group (pad col = 0.0): no uninitialized PSUM reads.
                    src = in_t[:, b0 - c0 : b0 - c0 + bn].rearrange(
                        "c (p k) -> c k p", k=4
                    )
                    for q in range(4):
                        nc.tensor.transpose(
                            ps_t[:, GP * q : GP * (q + 1)],
                            src[:, q, :],
                            id_t[:, :],
                        )
                    # mask = (max over class channels) > THRESHOLD
                    maxv = spool.tile([P, 4], F32, tag="mx")
                    red_in = ps_t[:, :].rearrange("p (g c) -> p g c", c=GP)[
                        :, :, 2:82
                    ]
                    nc.vector.reduce_max(
                        maxv[:, :], red_in, axis=mybir.AxisListType.X
                    )
                    mask = spool.tile([P, 4], F32, tag="mk")
                    nc.vector.tensor_scalar(
                        mask[:, :], maxv[:, :], THRESHOLD, None,
                        mybir.AluOpType.is_gt,
                    )
                    # Masked copies PSUM->dense staging (mask fused as the
                    # per-partition activation scale); 3 on ACT, 1 on DVE.
                    for q in range(4):
                        dst = stage[:, 348 * j + 87 * q : 348 * j + 87 * (q + 1)]
                        sq = ps_t[:, GP * q : GP * q + NCH]
                        mq = mask[:, q : q + 1]
                        if q == 3:
                            nc.vector.tensor_scalar(
                                dst, sq, mq, None, mybir.AluOpType.mult
                            )
                        else:
                            nc.scalar.mul(dst, sq, mq)
                    bi += 1
                # One batched output DMA per group, alternating rings.
                dram_view = out_d[g0 : g0 + 4 * P * L, :].rearrange(
                    "(j p k) c -> p j (k c)", j=L, k=4
                )
                sb_view = stage[:, :].rearrange("p (j x) -> p j x", x=348)
                out_eng = nc.gpsimd if (gi % 2) else nc.sync
                out_eng.dma_start(out=dram_view, in_=sb_view)
                gi += 1
        for p in reversed(pspools):
            p.release()


_NC_CACHE = None


def _split_multi_waits(nc):
    """Hoist all-but-one semaphore wait of every instruction into preceding
    single-wait NoOps on the same engine.

    The TPB instruction encoding has ONE sync-wait slot per instruction and
    this walrus build refuses to split multi-wait instructions itself
    ("Too many sync wait commands"). Engines process their instruction
    stream in order, so waiting via a chain of NoOps is equivalent.
    """
    f = nc.m.functions[0]
    for blk in f.blocks:
        targets = []
        for idx, inst in enumerate(blk.instructions):
            si = inst.sync_info
            if si is not None and len(si.on_wait) >= 2:
                targets.append((idx, inst))
        for idx, inst in reversed(targets):
            si = inst.sync_info
            waits = list(si.on_wait)
            inst.sync_info = mybir.SyncInfo(
                on_wait=[waits[-1]], on_update=list(si.on_update)
            )
            for w in reversed(waits[:-1]):
                nop = mybir.InstNoOp(
                    name=nc.get_next_instruction_name(),
                    bass_nofuse=True,
                    sync_info=mybir.SyncInfo(on_wait=[w], on_update=[]),
                )
                nop.engine = inst.engine
                nc.register_instruction(nop)
                blk.instructions.insert(idx, nop)


def _build_nc(reps=1):
    """Build the Bass program; reps>1 repeats the whole (idempotent) body —
    used only for marginal-time measurement in the test harness."""
    nc = bass.Bass("TRN2", target_bir_lowering=False, debug=False)
    pk = nc.dram_tensor("pk", [NR, TOTAL], F32, kind="ExternalInput")
    ident = nc.dram_tensor("ident", [NR, GP], F32, kind="ExternalInput")
    out = nc.dram_tensor("out", [TOTAL, 87], F32, kind="ExternalOutput")
    with tile.TileContext(nc) as tc:
        for _ in range(reps):
            _emit(tc, {"out": out[:, :]}, {"pk": pk[:, :], "ident": ident[:, :]})
    _split_multi_waits(nc)
    return nc


def _get_nc():
    global _NC_CACHE
    if _NC_CACHE is None:
        _NC_CACHE = _build_nc()
    return _NC_CACHE


def _coord_rows(h, stride):
    half = stride // 2
    xs = np.tile(np.arange(h, dtype=np.float32) * stride + half, h)
    ys = np.repeat(np.arange(h, dtype=np.float32) * stride + half, h)
    return xs, ys


def pack_inputs(inputs):
    """Build per-core input maps from the full (unsharded) input dict."""
    # [88, 88] permutation: transposes write the full 88-wide psum group;
    # input row 87 is zero, so the psum pad column lands as 0.0.
    ident = np.eye(GP, dtype=np.float32)
    # Per-level coordinate rows are identical across cores; compute once.
    coords = [_coord_rows(h, s) for h, s in zip(HS, STRIDES)]
    in_maps = []
    for b in range(B):
        pk = np.zeros((NR, TOTAL), np.float32)
        for li, h in enumerate(HS):
            o, hw = LEVEL_OFF[li], HWS[li]
            sl = slice(o, o + hw)
            pk[0, sl], pk[1, sl] = coords[li]
            pk[2:82, sl] = np.asarray(inputs[f"cls_{li}"][b]).reshape(C, hw)
            pk[82:86, sl] = np.asarray(inputs[f"bbox_{li}"][b]).reshape(4, hw)
            pk[86, sl] = np.asarray(inputs[f"ctr_{li}"][b]).reshape(hw)
        in_maps.append({"pk": pk, "ident": ident})
    return in_maps


def unpack_outputs(results):
    """results: list (per core) of {name: np.ndarray}. Returns tuple of 5."""
    outs = []
    for li, hw in enumerate(HWS):
        o = LEVEL_OFF[li]
        outs.append(
            np.stack([results[b]["out"][o : o + hw, :] for b in range(B)], axis=0)
        )
    return tuple(outs)


def kernel(**inputs):
    from concourse.bass_utils import run_bass_kernel_spmd

    in_maps = pack_inputs(inputs)
    nc = _get_nc()
    res = run_bass_kernel_spmd(nc, in_maps, core_ids=list(range(B)))
    return unpack_outputs(res.results)


# revision 31
# speedup vs baseline: 23.0743x; 1.4086x over previous
"""FCOS multi-stride filter kernel for Trainium2 (8 NeuronCores, batch-parallel).

Reference computation (per level l with stride s, per batch b):
    scores  = cls[b].reshape(C, H*W)          # natural NCHW layout
    mask    = (max_c scores > 0.5)            # per location
    out[loc] = [x*mask, y*mask, scores*mask, bbox*mask, ctr*mask]  # [HW, 87]

Strategy:
  - Host packs, per core (= batch sample), one [88, 34100] f32 tensor:
    rows 0:2 = precomputed pixel coords, 2:82 = cls, 82:86 = bbox, 86 = ctr,
    87 = zeros; columns = concatenated locations of all 5 pyramid levels.
  - Device (per core): for each block of 512 locations, 4 PE transposes
    (stride-4 interleave so each output partition holds 4 consecutive
    locations), DVE reduce_max + threshold compare for the mask, then 4
    masked copies (mask fused as the per-partition activation/tensor-scalar
    scale) from PSUM into a DENSE staging tile whose rows are 348-float
    contiguous runs. Up to 8 blocks share one staging tile and ONE ~1.4MB
    output DMA (26 DMAs total per core instead of 79 - per-DMA fixed cost
    dominated the measured runtime).
  - Output per core is [34100, 87]; host splits rows per level and stacks
    over the 8 cores.
"""

import numpy as np

import concourse.bass as bass
import concourse.tile as tile
from concourse import mybir

# Problem constants (hardcoded per harness contract)
B = 8
C = 80
HS = (160, 80, 40, 20, 10)
STRIDES = (8, 16, 32, 64, 128)
THRESHOLD = 0.5

HWS = [h * h for h in HS]
TOTAL = sum(HWS)  # 34100
LEVEL_OFF = [0]
for _hw in HWS:
    LEVEL_OFF.append(LEVEL_OFF[-1] + _hw)

NCH = 87  # 2 coord + 80 cls + 4 bbox + 1 ctr
NR = 88  # packed input rows (87 channels + 1 zero row so K=88 transposes are
         # true permutations for CoreSim and the psum pad column gets 0.0)
GP = 88  # per-group column pitch in psum/out tiles (pad to even for DVE 2x)
GW = 4 * GP  # 352 floats per psum/out tile row (4 locations x 88)
CHUNK = 12288  # input DMA chunk (columns)
MAXRUN = 16  # max blocks per staging group (SBUF budget)
F32 = mybir.dt.float32


def _blocks():
    """Yield (chunk_start, chunk_len, [group, ...]); each group is a list of
    consecutive (block_start, block_len) blocks sharing one output staging
    tile and one batched output DMA (runs of full 512-blocks, or one tail).

    Blocks are <=512 locations, never cross a chunk or level boundary, and
    are always divisible by 4.
    """
    chunk_bounds = list(range(0, TOTAL, CHUNK)) + [TOTAL]
    out = []
    for ci in range(len(chunk_bounds) - 1):
        c0, c1 = chunk_bounds[ci], chunk_bounds[ci + 1]
        groups = []
        for li in range(5):
            l0, l1 = LEVEL_OFF[li], LEVEL_OFF[li + 1]
            s = max(c0, l0)
            e = min(c1, l1)
            b0 = s
            run = []
            while b0 < e:
                bn = min(512, e - b0)
                assert bn % 4 == 0
                if bn == 512:
                    run.append((b0, bn))
                else:
                    if run:
                        groups.append(run)
                        run = []
                    groups.append([(b0, bn)])
                b0 += bn
            if run:
                groups.append(run)
        groups = [
            g[i : i + MAXRUN]
            for g in groups
            for i in range(0, len(g), MAXRUN)
        ]
        if groups:
            out.append((c0, c1 - c0, groups))
    return out


def _emit(tc, outs, ins):
    """Emit the per-core Bass program under a TileContext."""
    nc = tc.nc
    pk = ins["pk"]        # [87, TOTAL] dram
    ident = ins["ident"]  # [87, 87] dram
    out_d = outs["out"]   # [TOTAL, 87] dram

    NPS = 4  # psum ping-pong depth (deterministic tenancy: block i evicts i-4)

    with (
        tc.tile_pool(name="const", bufs=1) as cpool,
        tc.tile_pool(name="inp", bufs=2) as ipool,
        tc.tile_pool(name="stg", bufs=3) as stpool,
        tc.tile_pool(name="small", bufs=8) as spool,
    ):
        pspools = [
            tc.alloc_tile_pool(name=f"ps{i}", bufs=1, space="PSUM")
            for i in range(NPS)
        ]
        id_t = cpool.tile([NR, GP], F32)
        nc.sync.dma_start(out=id_t[:, :], in_=ident[:, :])

        bi = 0
        ci = 0
        gi = 0
        for c0, cn, groups in _blocks():
            in_t = ipool.tile([NR, cn], F32, tag="in")
            # Spread input chunks over the SP HWDGE ring and the SWDGE
            # (gpsimd) ring so no single descriptor ring serializes all
            # transfers; ACT's ring is kept free for compute.
            in_eng = nc.gpsimd if ci == 0 else nc.sync
            in_eng.dma_start(out=in_t[:, :], in_=pk[:, c0 : c0 + cn])
            ci += 1

            for group in groups:
                L = len(group)
                P = group[0][1] // 4
                g0 = group[0][0]
                # Dense staging: location (g0 + 4P*j + 4p + q), channel c at
                # stage[p, 348*j + 87*q + c] -- contiguous 1392B runs that
                # batch L blocks into ONE output DMA (3-dim balanced APs).
                stage = stpool.tile([P, 348 * L], F32, tag="st")
                for j, (b0, bn) in enumerate(group):
                    ps_t = pspools[bi % NPS].tile(
                        [P, GW], F32, tag="ps", name=f"ps_t{bi % NPS}"
                    )
                    # [NR, bn] -> [NR, 4, P] view: location = b0 + 4*p + q
                    # The identity is the [88, 88] permutation; input row 87
                    # is zero, so each transpose writes the full 88-wide
                    # group (pad col = 0.0): no uninitialized PSUM reads.
                    src = in_t[:, b0 - c0 : b0 - c0 + bn].rearrange(
                        "c (p k) -> c k p", k=4
                    )
                    for q in range(4):
                        nc.tensor.transpose(
                            ps_t[:, GP * q : GP * (q + 1)],
                            src[:, q, :],
                            id_t[:, :],
                        )
                    # mask = (max over class channels) > THRESHOLD
                    maxv = spool.tile([P, 4], F32, tag="mx")
                    red_in = ps_t[:, :].rearrange("p (g c) -> p g c", c=GP)[
                        :, :, 2:82
                    ]
                    nc.vector.reduce_max(
                        maxv[:, :], red_in, axis=mybir.AxisListType.X
                    )
                    mask = spool.tile([P, 4], F32, tag="mk")
                    nc.vector.tensor_scalar(
                        mask[:, :], maxv[:, :], THRESHOLD, None,
                        mybir.AluOpType.is_gt,
                    )
                    # Masked copies PSUM->dense staging (mask fused as the
                    # per-partition activation scale). Alternate 3ACT/1DVE and
                    # 2ACT/2DVE per block: ~2.5/1.5 average balances the two
                    # engines' measured throughput.
                    n_act = 3 if (bi % 2 == 0) else 2
                    for q in range(4):
                        dst = stage[:, 348 * j + 87 * q : 348 * j + 87 * (q + 1)]
                        sq = ps_t[:, GP * q : GP * q + NCH]
                        mq = mask[:, q : q + 1]
                        if q < n_act:
                            nc.scalar.mul(dst, sq, mq)
                        else:
                            nc.vector.tensor_scalar(
                                dst, sq, mq, None, mybir.AluOpType.mult
                            )
                    bi += 1
                # One batched output DMA per group, alternating rings.
                dram_view = out_d[g0 : g0 + 4 * P * L, :].rearrange(
                    "(j p k) c -> p j (k c)", j=L, k=4
                )
                sb_view = stage[:, :].rearrange("p (j x) -> p j x", x=348)
                out_eng = nc.gpsimd if (gi % 2) else nc.sync
                out_eng.dma_start(out=dram_view, in_=sb_view)
                gi += 1
        for p in reversed(pspools):
            p.release()


_NC_CACHE = None


def _split_multi_waits(nc):
    """Hoist all-but-one semaphore wait of every instruction into preceding
    single-wait NoOps on the same engine.

    The TPB instruction encoding has ONE sync-wait slot per instruction and
    this walrus build refuses to split multi-wait instructions itself
    ("Too many sync wait commands"). Engines process their instruction
    stream in order, so waiting via a chain of NoOps is equivalent.
    """
    f = nc.m.functions[0]
    for blk in f.blocks:
        targets = []
        for idx, inst in enumerate(blk.instructions):
            si = inst.sync_info
            if si is not None and len(si.on_wait) >= 2:
                targets.append((idx, inst))
        for idx, inst in reversed(targets):
            si = inst.sync_info
            waits = list(si.on_wait)
            inst.sync_info = mybir.SyncInfo(
                on_wait=[waits[-1]], on_update=list(si.on_update)
            )
            for w in reversed(waits[:-1]):
                nop = mybir.InstNoOp(
                    name=nc.get_next_instruction_name(),
                    bass_nofuse=True,
                    sync_info=mybir.SyncInfo(on_wait=[w], on_update=[]),
                )
                nop.engine = inst.engine
                nc.register_instruction(nop)
                blk.instructions.insert(idx, nop)


def _build_nc(reps=1):
    """Build the Bass program; reps>1 repeats the whole (idempotent) body —
    used only for marginal-time measurement in the test harness."""
    nc = bass.Bass("TRN2", target_bir_lowering=False, debug=False)
    pk = nc.dram_tensor("pk", [NR, TOTAL], F32, kind="ExternalInput")
    ident = nc.dram_tensor("ident", [NR, GP], F32, kind="ExternalInput")
    out = nc.dram_tensor("out", [TOTAL, 87], F32, kind="ExternalOutput")
    with tile.TileContext(nc) as tc:
        for _ in range(reps):
            _emit(tc, {"out": out[:, :]}, {"pk": pk[:, :], "ident": ident[:, :]})
    _split_multi_waits(nc)
    return nc


def _get_nc():
    global _NC_CACHE
    if _NC_CACHE is None:
        _NC_CACHE = _build_nc()
    return _NC_CACHE


def _coord_rows(h, stride):
    half = stride // 2
    xs = np.tile(np.arange(h, dtype=np.float32) * stride + half, h)
    ys = np.repeat(np.arange(h, dtype=np.float32) * stride + half, h)
    return xs, ys


def pack_inputs(inputs):
    """Build per-core input maps from the full (unsharded) input dict."""
    # [88, 88] permutation: transposes write the full 88-wide psum group;
    # input row 87 is zero, so the psum pad column lands as 0.0.
    ident = np.eye(GP, dtype=np.float32)
    # Per-level coordinate rows are identical across cores; compute once.
    coords = [_coord_rows(h, s) for h, s in zip(HS, STRIDES)]
    in_maps = []
    for b in range(B):
        pk = np.zeros((NR, TOTAL), np.float32)
        for li, h in enumerate(HS):
            o, hw = LEVEL_OFF[li], HWS[li]
            sl = slice(o, o + hw)
            pk[0, sl], pk[1, sl] = coords[li]
            pk[2:82, sl] = np.asarray(inputs[f"cls_{li}"][b]).reshape(C, hw)
            pk[82:86, sl] = np.asarray(inputs[f"bbox_{li}"][b]).reshape(4, hw)
            pk[86, sl] = np.asarray(inputs[f"ctr_{li}"][b]).reshape(hw)
        in_maps.append({"pk": pk, "ident": ident})
    return in_maps


def unpack_outputs(results):
    """results: list (per core) of {name: np.ndarray}. Returns tuple of 5."""
    outs = []
    for li, hw in enumerate(HWS):
        o = LEVEL_OFF[li]
        outs.append(
            np.stack([results[b]["out"][o : o + hw, :] for b in range(B)], axis=0)
        )
    return tuple(outs)


def kernel(**inputs):
    from concourse.bass_utils import run_bass_kernel_spmd

    in_maps = pack_inputs(inputs)
    nc = _get_nc()
    res = run_bass_kernel_spmd(nc, in_maps, core_ids=list(range(B)))
    return unpack_outputs(res.results)
